# revision 7
# baseline (speedup 1.0000x reference)
"""Trainium2 Bass kernel for similarity-matrix penalty (gnn message passing).

penalty = sum_{b,k} S[b,k] * || P[i_b] - P[j_{b,k}] ||_2

Strategy (per the batch-sharding hint):
  - Shard the batch dim B=4096 across 8 cores (512 b's each); P is
    replicated to every core's HBM.
  - Per core, 4 chunks of 128 b's. Chunk layout: partition p <-> b,
    free dim = 65 row-slots x 128 d (slot 0 = P[i_b], slots 1..64 =
    P[j_{b,k}]).
  - ONE indirect DMA per chunk gathers all 65 rows per partition
    (8320 indices, ~3.8us of Q7 desc-gen) instead of one call per k
    (65 calls x ~1.8us fixed each). The dest AP MUST be 2D
    ([128, KI*D]); a 3D dest AP makes the SWDGE ucode emit only KI
    descriptors. The gather also casts fp32->bf16 in the SDMA
    datapath (SWDGE cast), halving SBUF write traffic and enabling
    the DVE 2x packed mode for the subtract.
  - diff on DVE (bf16 tensor_tensor subtract with a stride-0
    broadcast AP of row slot 0, in-place).
  - square+reduce split between engines to balance the pipeline:
    chunks in ACT_CHUNKS use per-k ACT Square with fp32 accum_out
    (64 small insts); the rest use one ACT square + one DVE grouped
    tensor_reduce.
  - sqrt on ACT, multiply by S, free-dim reduce -> [128,1] per core.
  - Host adds the 8x128 partials.
"""

import contextlib

import numpy as np

import concourse.bass as bass
import concourse.mybir as mybir
import concourse.tile as tile
from concourse import bacc
from concourse import bass_utils

N_ROWS = 500000
D = 128
B = 4096
K = 64
N_CORES = 8
B_PER_CORE = B // N_CORES      # 512
CHUNKS = 4
BC = B_PER_CORE // CHUNKS      # 128 b's per chunk (one per partition)
KI = K + 1                     # row slots per partition per chunk (i + 64 j's)
FP32 = mybir.dt.float32
BF16 = mybir.dt.bfloat16
I32 = mybir.dt.int32

# chunk indices whose square+reduce runs fused on ACT (per-k accum);
# the others use ACT square + DVE grouped reduce
ACT_CHUNKS = (1, 3)
WP_BUFS = 3
GATHER_DT = BF16
TABLE_DT = FP32       # dtype of P as staged in DRAM (host pre-casts if not fp32)
GATHER_ONLY = False   # diagnostic: skip all per-chunk compute
K_SPLIT = 1           # split each chunk's K dim into this many gathers
REDUCE_TREE = False   # pairwise-add d-halves (bf16 2x) before the reduce

_PROGRAM_CACHE = {}


def _build_program(repeat=1):
    nc = bacc.Bacc(
        "TRN2",
        debug=False,
        enable_asserts=False,
        target_bir_lowering=False,
        num_devices=N_CORES,
        dynamic_dma_scratch_size=65536,
    )

    P_d = nc.dram_tensor("P", [N_ROWS, D], TABLE_DT, kind="ExternalInput")
    n_idx = CHUNKS * K_SPLIT * (K // K_SPLIT + 1)
    idx_d = nc.dram_tensor("idxIJ", [128, n_idx], I32, kind="ExternalInput")
    S_d = nc.dram_tensor("S", [128, CHUNKS * K], FP32, kind="ExternalInput")
    out_d = nc.dram_tensor("out", [128, 1], FP32, kind="ExternalOutput")

    with tile.TileContext(nc) as tc:
        with (
            tc.tile_pool(name="persist", bufs=1) as pp,
            tc.tile_pool(name="work", bufs=WP_BUFS) as wp,
        ):
            idx_sb = pp.tile([128, n_idx], I32)
            nc.sync.dma_start(idx_sb[:], idx_d[:, :])
            S_sb = pp.tile([128, CHUNKS * K], FP32)
            nc.sync.dma_start(S_sb[:], S_d[:, :])

            norm2 = pp.tile([128, CHUNKS * K], FP32)

            KH = K // K_SPLIT          # j rows per gather
            KHI = KH + 1               # +1 for the i row
            NCH = CHUNKS * K_SPLIT     # total gathers per rep
            # repeat>1 benches the body in a hardware loop: constant program
            # size for any R (the body is idempotent, norm2 rewritten each
            # iteration), so test.py's repeat-difference can use R >> 1.
            rep_ctx = tc.For_i(0, repeat) if repeat > 1 else contextlib.nullcontext(0)
            with rep_ctx:
              for ch in range(NCH):
                # big[p, 0, :] <- P[i_b]; big[p, 1+kk, :] <- P[j_{b,k}]
                # for b = (ch//K_SPLIT)*128+p, k = (ch%K_SPLIT)*KH+kk,
                # all in ONE indirect DMA, cast by the SDMA datapath.
                big = wp.tile([128, KHI, D], GATHER_DT)
                nc.gpsimd.indirect_dma_start(
                    out=big[:, :, :].rearrange("p a b -> p (a b)"),
                    out_offset=None,
                    in_=P_d[:, :],
                    in_offset=bass.IndirectOffsetOnAxis(
                        ap=idx_sb[:, ch * KHI : (ch + 1) * KHI], axis=0
                    ),
                )
                if GATHER_ONLY:
                    continue
                # diff on DVE (broadcast row slot 0 along k), in place
                nc.vector.tensor_tensor(
                    out=big[:, 1:KHI, :],
                    in0=big[:, 1:KHI, :],
                    in1=big[:, 0:1, :].to_broadcast((128, KH, D)),
                    op=mybir.AluOpType.subtract,
                )
                if ch in ACT_CHUNKS:
                    # fused square+sum on ACT, one inst per k
                    for k in range(KH):
                        nc.scalar.activation(
                            out=big[:, 1 + k, :],
                            in_=big[:, 1 + k, :],
                            func=mybir.ActivationFunctionType.Square,
                            accum_out=norm2[:, ch * KH + k : ch * KH + k + 1],
                        )
                elif REDUCE_TREE:
                    nc.scalar.square(big[:, 1:KHI, :], big[:, 1:KHI, :])
                    half = wp.tile([128, KH, D // 2], GATHER_DT)
                    nc.vector.tensor_tensor(
                        out=half[:],
                        in0=big[:, 1:KHI, 0 : D // 2],
                        in1=big[:, 1:KHI, D // 2 : D],
                        op=mybir.AluOpType.add,
                    )
                    nc.vector.reduce_sum(
                        norm2[:, ch * KH : (ch + 1) * KH],
                        half[:],
                        axis=mybir.AxisListType.X,
                    )
                else:
                    # square on ACT, grouped free-dim reduce on DVE
                    nc.scalar.square(big[:, 1:KHI, :], big[:, 1:KHI, :])
                    nc.vector.reduce_sum(
                        norm2[:, ch * KH : (ch + 1) * KH],
                        big[:, 1:KHI, :],
                        axis=mybir.AxisListType.X,
                    )

            if GATHER_ONLY:
                nc.vector.memset(norm2[:], 0.0)
            # finals: sqrt -> *S -> free-dim reduce -> DRAM
            norms = pp.tile([128, CHUNKS * K], FP32)
            nc.scalar.sqrt(norms[:], norm2[:])
            weighted = pp.tile([128, CHUNKS * K], FP32)
            nc.vector.tensor_tensor(
                out=weighted[:], in0=norms[:], in1=S_sb[:], op=mybir.AluOpType.mult
            )
            res = pp.tile([128, 1], FP32)
            nc.vector.reduce_sum(res[:], weighted[:], axis=mybir.AxisListType.X)
            nc.sync.dma_start(out_d[:, :], res[:])

    nc.compile()
    return nc


def get_program(repeat=1):
    if repeat not in _PROGRAM_CACHE:
        _PROGRAM_CACHE[repeat] = _build_program(repeat)
    return _PROGRAM_CACHE[repeat]


def make_in_maps(P, i_indices, j_indices, S_vals):
    P = np.ascontiguousarray(np.asarray(P, dtype=np.float32))
    if TABLE_DT is not FP32:
        P = P.astype(mybir.dt.np(TABLE_DT))
    i_idx = np.asarray(i_indices).astype(np.int32)
    j_idx = np.asarray(j_indices).astype(np.int32)
    S = np.asarray(S_vals, dtype=np.float32)
    in_maps = []
    for core in range(N_CORES):
        b0 = core * B_PER_CORE
        i_c = i_idx[b0 : b0 + B_PER_CORE]            # [512]
        j_c = j_idx[b0 : b0 + B_PER_CORE]            # [512, 64]
        S_c = S[b0 : b0 + B_PER_CORE]                # [512, 64]
        # sort each b's (j, S) pairs by j: the penalty is a flat sum over
        # (b, k), so any per-b co-permutation is exact; ascending row ids
        # give the gather's descriptors better HBM locality.
        order = np.argsort(j_c, axis=1)
        j_c = np.take_along_axis(j_c, order, axis=1)
        S_c = np.take_along_axis(S_c, order, axis=1)
        # per half-chunk (c, h): slots [i_b, j_{b, h*KH .. h*KH+KH-1}];
        # partition p <-> b = c*128+p
        KH = K // K_SPLIT
        j_r = j_c.reshape(CHUNKS, BC, K_SPLIT, KH)
        i_r = np.broadcast_to(
            i_c.reshape(CHUNKS, BC)[:, :, None, None], (CHUNKS, BC, K_SPLIT, 1)
        )
        ij = np.concatenate([i_r, j_r], axis=3)          # [CHUNKS, BC, K_SPLIT, KH+1]
        idxIJ = np.ascontiguousarray(
            ij.transpose(1, 0, 2, 3).reshape(BC, CHUNKS * K_SPLIT * (KH + 1))
        )
        S_arr = np.ascontiguousarray(
            S_c.reshape(CHUNKS, BC, K_SPLIT, KH).transpose(1, 0, 2, 3)
            .reshape(BC, CHUNKS * K)
        )
        in_maps.append({"P": P, "idxIJ": idxIJ, "S": S_arr})
    return in_maps


def run_hw(in_maps, trace=False, repeat=1):
    nc = get_program(repeat)
    return bass_utils.run_bass_kernel_spmd(
        nc,
        in_maps,
        core_ids=list(range(N_CORES)),
        trace=trace,
    )


def kernel(P, i_indices, j_indices, S_vals):
    in_maps = make_in_maps(P, i_indices, j_indices, S_vals)
    res = run_hw(in_maps, trace=False)
    total = 0.0
    for core in range(N_CORES):
        total += float(np.asarray(res.results[core]["out"], dtype=np.float64).sum())
    return np.float32(total)



# revision 8
# speedup vs baseline: 1.4452x; 1.4452x over previous
"""Trainium2 Bass kernel for similarity-matrix penalty (gnn message passing).

penalty = sum_{b,k} S[b,k] * || P[i_b] - P[j_{b,k}] ||_2

Strategy (per the batch-sharding hint):
  - Shard the batch dim B=4096 across 8 cores (512 b's each); P is
    replicated to every core's HBM.
  - Per core, 4 chunks of 128 b's. Chunk layout: partition p <-> b,
    free dim = 65 row-slots x 128 d (slot 0 = P[i_b], slots 1..64 =
    P[j_{b,k}]).
  - ONE indirect DMA per chunk gathers all 65 rows per partition
    (8320 indices, ~3.8us of Q7 desc-gen) instead of one call per k
    (65 calls x ~1.8us fixed each). The dest AP MUST be 2D
    ([128, KI*D]); a 3D dest AP makes the SWDGE ucode emit only KI
    descriptors. The gather also casts fp32->bf16 in the SDMA
    datapath (SWDGE cast), halving SBUF write traffic and enabling
    the DVE 2x packed mode for the subtract.
  - diff on DVE (bf16 tensor_tensor subtract with a stride-0
    broadcast AP of row slot 0, in-place).
  - square+reduce split between engines to balance the pipeline:
    chunks in ACT_CHUNKS use per-k ACT Square with fp32 accum_out
    (64 small insts); the rest use one ACT square + one DVE grouped
    tensor_reduce.
  - sqrt on ACT, multiply by S, free-dim reduce -> [128,1] per core.
  - Host adds the 8x128 partials.
"""

import contextlib

import numpy as np

import concourse.bass as bass
import concourse.mybir as mybir
import concourse.tile as tile
from concourse import bacc
from concourse import bass_utils

N_ROWS = 500000
D = 128
B = 4096
K = 64
N_CORES = 8
B_PER_CORE = B // N_CORES      # 512
CHUNKS = 4
BC = B_PER_CORE // CHUNKS      # 128 b's per chunk (one per partition)
KI = K + 1                     # row slots per partition per chunk (i + 64 j's)
FP32 = mybir.dt.float32
BF16 = mybir.dt.bfloat16
I32 = mybir.dt.int32

# chunk indices whose square+reduce runs fused on ACT (per-k accum);
# the others use ACT square + DVE grouped reduce
ACT_CHUNKS = ()
WP_BUFS = 6
GATHER_DT = BF16
TABLE_DT = FP32       # dtype of P as staged in DRAM (host pre-casts if not fp32)
GATHER_ONLY = False   # diagnostic: skip all per-chunk compute
K_SPLIT = 2           # split each chunk's K dim into this many gathers
REDUCE_TREE = False   # pairwise-add d-halves (bf16 2x) before the reduce

_PROGRAM_CACHE = {}


def _build_program(repeat=1):
    nc = bacc.Bacc(
        "TRN2",
        debug=False,
        enable_asserts=False,
        target_bir_lowering=False,
        num_devices=N_CORES,
        dynamic_dma_scratch_size=65536,
    )

    P_d = nc.dram_tensor("P", [N_ROWS, D], TABLE_DT, kind="ExternalInput")
    n_idx = CHUNKS * K_SPLIT * (K // K_SPLIT + 1)
    idx_d = nc.dram_tensor("idxIJ", [128, n_idx], I32, kind="ExternalInput")
    S_d = nc.dram_tensor("S", [128, CHUNKS * K], FP32, kind="ExternalInput")
    out_d = nc.dram_tensor("out", [128, 1], FP32, kind="ExternalOutput")

    with tile.TileContext(nc) as tc:
        with (
            tc.tile_pool(name="persist", bufs=1) as pp,
            tc.tile_pool(name="work", bufs=WP_BUFS) as wp,
        ):
            idx_sb = pp.tile([128, n_idx], I32)
            nc.sync.dma_start(idx_sb[:], idx_d[:, :])
            S_sb = pp.tile([128, CHUNKS * K], FP32)
            nc.sync.dma_start(S_sb[:], S_d[:, :])

            norm2 = pp.tile([128, CHUNKS * K], FP32)

            KH = K // K_SPLIT          # j rows per gather
            KHI = KH + 1               # +1 for the i row
            NCH = CHUNKS * K_SPLIT     # total gathers per rep
            # repeat>1 benches the body in a hardware loop: constant program
            # size for any R (the body is idempotent, norm2 rewritten each
            # iteration), so test.py's repeat-difference can use R >> 1.
            rep_ctx = tc.For_i(0, repeat) if repeat > 1 else contextlib.nullcontext(0)
            with rep_ctx:
              for ch in range(NCH):
                # big[p, 0, :] <- P[i_b]; big[p, 1+kk, :] <- P[j_{b,k}]
                # for b = (ch//K_SPLIT)*128+p, k = (ch%K_SPLIT)*KH+kk,
                # all in ONE indirect DMA, cast by the SDMA datapath.
                big = wp.tile([128, KHI, D], GATHER_DT)
                nc.gpsimd.indirect_dma_start(
                    out=big[:, :, :].rearrange("p a b -> p (a b)"),
                    out_offset=None,
                    in_=P_d[:, :],
                    in_offset=bass.IndirectOffsetOnAxis(
                        ap=idx_sb[:, ch * KHI : (ch + 1) * KHI], axis=0
                    ),
                )
                if GATHER_ONLY:
                    continue
                # diff on DVE (broadcast row slot 0 along k), in place
                nc.vector.tensor_tensor(
                    out=big[:, 1:KHI, :],
                    in0=big[:, 1:KHI, :],
                    in1=big[:, 0:1, :].to_broadcast((128, KH, D)),
                    op=mybir.AluOpType.subtract,
                )
                if ch in ACT_CHUNKS:
                    # fused square+sum on ACT, one inst per k
                    for k in range(KH):
                        nc.scalar.activation(
                            out=big[:, 1 + k, :],
                            in_=big[:, 1 + k, :],
                            func=mybir.ActivationFunctionType.Square,
                            accum_out=norm2[:, ch * KH + k : ch * KH + k + 1],
                        )
                elif REDUCE_TREE:
                    nc.scalar.square(big[:, 1:KHI, :], big[:, 1:KHI, :])
                    half = wp.tile([128, KH, D // 2], GATHER_DT)
                    nc.vector.tensor_tensor(
                        out=half[:],
                        in0=big[:, 1:KHI, 0 : D // 2],
                        in1=big[:, 1:KHI, D // 2 : D],
                        op=mybir.AluOpType.add,
                    )
                    nc.vector.reduce_sum(
                        norm2[:, ch * KH : (ch + 1) * KH],
                        half[:],
                        axis=mybir.AxisListType.X,
                    )
                else:
                    # square on ACT, grouped free-dim reduce on DVE
                    nc.scalar.square(big[:, 1:KHI, :], big[:, 1:KHI, :])
                    nc.vector.reduce_sum(
                        norm2[:, ch * KH : (ch + 1) * KH],
                        big[:, 1:KHI, :],
                        axis=mybir.AxisListType.X,
                    )

            if GATHER_ONLY:
                nc.vector.memset(norm2[:], 0.0)
            # finals: sqrt -> *S -> free-dim reduce -> DRAM
            norms = pp.tile([128, CHUNKS * K], FP32)
            nc.scalar.sqrt(norms[:], norm2[:])
            weighted = pp.tile([128, CHUNKS * K], FP32)
            nc.vector.tensor_tensor(
                out=weighted[:], in0=norms[:], in1=S_sb[:], op=mybir.AluOpType.mult
            )
            res = pp.tile([128, 1], FP32)
            nc.vector.reduce_sum(res[:], weighted[:], axis=mybir.AxisListType.X)
            nc.sync.dma_start(out_d[:, :], res[:])

    nc.compile()
    return nc


def get_program(repeat=1):
    if repeat not in _PROGRAM_CACHE:
        _PROGRAM_CACHE[repeat] = _build_program(repeat)
    return _PROGRAM_CACHE[repeat]


def make_in_maps(P, i_indices, j_indices, S_vals):
    P = np.ascontiguousarray(np.asarray(P, dtype=np.float32))
    if TABLE_DT is not FP32:
        P = P.astype(mybir.dt.np(TABLE_DT))
    i_idx = np.asarray(i_indices).astype(np.int32)
    j_idx = np.asarray(j_indices).astype(np.int32)
    S = np.asarray(S_vals, dtype=np.float32)
    in_maps = []
    for core in range(N_CORES):
        b0 = core * B_PER_CORE
        i_c = i_idx[b0 : b0 + B_PER_CORE]            # [512]
        j_c = j_idx[b0 : b0 + B_PER_CORE]            # [512, 64]
        S_c = S[b0 : b0 + B_PER_CORE]                # [512, 64]
        # sort each b's (j, S) pairs by j: the penalty is a flat sum over
        # (b, k), so any per-b co-permutation is exact; ascending row ids
        # give the gather's descriptors better HBM locality.
        order = np.argsort(j_c, axis=1)
        j_c = np.take_along_axis(j_c, order, axis=1)
        S_c = np.take_along_axis(S_c, order, axis=1)
        # per half-chunk (c, h): slots [i_b, j_{b, h*KH .. h*KH+KH-1}];
        # partition p <-> b = c*128+p
        KH = K // K_SPLIT
        j_r = j_c.reshape(CHUNKS, BC, K_SPLIT, KH)
        i_r = np.broadcast_to(
            i_c.reshape(CHUNKS, BC)[:, :, None, None], (CHUNKS, BC, K_SPLIT, 1)
        )
        ij = np.concatenate([i_r, j_r], axis=3)          # [CHUNKS, BC, K_SPLIT, KH+1]
        idxIJ = np.ascontiguousarray(
            ij.transpose(1, 0, 2, 3).reshape(BC, CHUNKS * K_SPLIT * (KH + 1))
        )
        S_arr = np.ascontiguousarray(
            S_c.reshape(CHUNKS, BC, K_SPLIT, KH).transpose(1, 0, 2, 3)
            .reshape(BC, CHUNKS * K)
        )
        in_maps.append({"P": P, "idxIJ": idxIJ, "S": S_arr})
    return in_maps


def run_hw(in_maps, trace=False, repeat=1):
    nc = get_program(repeat)
    return bass_utils.run_bass_kernel_spmd(
        nc,
        in_maps,
        core_ids=list(range(N_CORES)),
        trace=trace,
    )


def kernel(P, i_indices, j_indices, S_vals):
    in_maps = make_in_maps(P, i_indices, j_indices, S_vals)
    res = run_hw(in_maps, trace=False)
    total = 0.0
    for core in range(N_CORES):
        total += float(np.asarray(res.results[core]["out"], dtype=np.float64).sum())
    return np.float32(total)



# revision 9
# speedup vs baseline: 1.4565x; 1.0079x over previous
"""Trainium2 Bass kernel for similarity-matrix penalty (gnn message passing).

penalty = sum_{b,k} S[b,k] * || P[i_b] - P[j_{b,k}] ||_2

Strategy (per the batch-sharding hint):
  - Shard the batch dim B=4096 across 8 cores (512 b's each); P is
    replicated to every core's HBM.
  - Per core, 4 chunks of 128 b's, each split into K_SPLIT=2 gathers.
    Gather layout: partition p <-> b, free dim = 33 row-slots x 128 d
    (slot 0 = P[i_b], slots 1..32 = P[j_{b,k}] for half the k's).
  - ONE indirect DMA per gather fetches all 33 rows per partition
    (4224 indices) instead of one call per k. The dest AP MUST be 2D
    ([128, KHI*D]); a 3D dest AP makes the SWDGE ucode emit only KHI
    descriptors. The gather casts fp32->bf16 in the SDMA datapath
    (SWDGE cast), halving SBUF write traffic and enabling the DVE 2x
    packed mode for the subtract.
  - K_SPLIT=2 x WP_BUFS=6 keeps ~5 indirect DMAs in flight, spreading
    descriptors across more SDMA engine rings: measured 100us -> 71us
    per iteration (HW, For_i repeat-difference, R=1001).
  - Each b's (j, S) pairs are sorted by j on the host: the penalty is
    a flat sum over (b, k), so any per-b co-permutation is exact, and
    ascending row addresses give the gather better HBM locality (~2us).
  - diff on DVE (bf16 tensor_tensor subtract with a stride-0
    broadcast AP of row slot 0, in-place), ACT square, DVE grouped
    tensor_reduce -> norm2; sqrt on ACT, multiply by S, free-dim
    reduce -> [128,1] per core. Host adds the 8x128 partials.
  - Measured: 70.7us/iter on 8 trn2 cores, rel err 4.0e-05 (fp32
    table; bf16 only in the SDMA cast + on-chip compute).
"""

import contextlib

import numpy as np

import concourse.bass as bass
import concourse.mybir as mybir
import concourse.tile as tile
from concourse import bacc
from concourse import bass_utils

N_ROWS = 500000
D = 128
B = 4096
K = 64
N_CORES = 8
B_PER_CORE = B // N_CORES      # 512
CHUNKS = 4
BC = B_PER_CORE // CHUNKS      # 128 b's per chunk (one per partition)
KI = K + 1                     # row slots per partition per chunk (i + 64 j's)
FP32 = mybir.dt.float32
BF16 = mybir.dt.bfloat16
I32 = mybir.dt.int32

# chunk indices whose square+reduce runs fused on ACT (per-k accum);
# the others use ACT square + DVE grouped reduce
ACT_CHUNKS = ()
WP_BUFS = 6
GATHER_DT = BF16
TABLE_DT = FP32       # dtype of P as staged in DRAM (host pre-casts if not fp32)
GATHER_ONLY = False   # diagnostic: skip all per-chunk compute
K_SPLIT = 2           # split each chunk's K dim into this many gathers
REDUCE_TREE = False   # pairwise-add d-halves (bf16 2x) before the reduce

_PROGRAM_CACHE = {}


def _build_program(repeat=1):
    nc = bacc.Bacc(
        "TRN2",
        debug=False,
        enable_asserts=False,
        target_bir_lowering=False,
        num_devices=N_CORES,
        dynamic_dma_scratch_size=65536,
    )

    P_d = nc.dram_tensor("P", [N_ROWS, D], TABLE_DT, kind="ExternalInput")
    n_idx = CHUNKS * K_SPLIT * (K // K_SPLIT + 1)
    idx_d = nc.dram_tensor("idxIJ", [128, n_idx], I32, kind="ExternalInput")
    S_d = nc.dram_tensor("S", [128, CHUNKS * K], FP32, kind="ExternalInput")
    out_d = nc.dram_tensor("out", [128, 1], FP32, kind="ExternalOutput")

    with tile.TileContext(nc) as tc:
        with (
            tc.tile_pool(name="persist", bufs=1) as pp,
            tc.tile_pool(name="work", bufs=WP_BUFS) as wp,
        ):
            idx_sb = pp.tile([128, n_idx], I32)
            nc.sync.dma_start(idx_sb[:], idx_d[:, :])
            S_sb = pp.tile([128, CHUNKS * K], FP32)
            nc.sync.dma_start(S_sb[:], S_d[:, :])

            norm2 = pp.tile([128, CHUNKS * K], FP32)

            KH = K // K_SPLIT          # j rows per gather
            KHI = KH + 1               # +1 for the i row
            NCH = CHUNKS * K_SPLIT     # total gathers per rep
            # repeat>1 benches the body in a hardware loop: constant program
            # size for any R (the body is idempotent, norm2 rewritten each
            # iteration), so test.py's repeat-difference can use R >> 1.
            rep_ctx = tc.For_i(0, repeat) if repeat > 1 else contextlib.nullcontext(0)
            with rep_ctx:
              for ch in range(NCH):
                # big[p, 0, :] <- P[i_b]; big[p, 1+kk, :] <- P[j_{b,k}]
                # for b = (ch//K_SPLIT)*128+p, k = (ch%K_SPLIT)*KH+kk,
                # all in ONE indirect DMA, cast by the SDMA datapath.
                big = wp.tile([128, KHI, D], GATHER_DT)
                nc.gpsimd.indirect_dma_start(
                    out=big[:, :, :].rearrange("p a b -> p (a b)"),
                    out_offset=None,
                    in_=P_d[:, :],
                    in_offset=bass.IndirectOffsetOnAxis(
                        ap=idx_sb[:, ch * KHI : (ch + 1) * KHI], axis=0
                    ),
                )
                if GATHER_ONLY:
                    continue
                # diff on DVE (broadcast row slot 0 along k), in place
                nc.vector.tensor_tensor(
                    out=big[:, 1:KHI, :],
                    in0=big[:, 1:KHI, :],
                    in1=big[:, 0:1, :].to_broadcast((128, KH, D)),
                    op=mybir.AluOpType.subtract,
                )
                if ch in ACT_CHUNKS:
                    # fused square+sum on ACT, one inst per k
                    for k in range(KH):
                        nc.scalar.activation(
                            out=big[:, 1 + k, :],
                            in_=big[:, 1 + k, :],
                            func=mybir.ActivationFunctionType.Square,
                            accum_out=norm2[:, ch * KH + k : ch * KH + k + 1],
                        )
                elif REDUCE_TREE:
                    nc.scalar.square(big[:, 1:KHI, :], big[:, 1:KHI, :])
                    half = wp.tile([128, KH, D // 2], GATHER_DT)
                    nc.vector.tensor_tensor(
                        out=half[:],
                        in0=big[:, 1:KHI, 0 : D // 2],
                        in1=big[:, 1:KHI, D // 2 : D],
                        op=mybir.AluOpType.add,
                    )
                    nc.vector.reduce_sum(
                        norm2[:, ch * KH : (ch + 1) * KH],
                        half[:],
                        axis=mybir.AxisListType.X,
                    )
                else:
                    # square on ACT, grouped free-dim reduce on DVE
                    nc.scalar.square(big[:, 1:KHI, :], big[:, 1:KHI, :])
                    nc.vector.reduce_sum(
                        norm2[:, ch * KH : (ch + 1) * KH],
                        big[:, 1:KHI, :],
                        axis=mybir.AxisListType.X,
                    )

            if GATHER_ONLY:
                nc.vector.memset(norm2[:], 0.0)
            # finals: sqrt -> *S -> free-dim reduce -> DRAM
            norms = pp.tile([128, CHUNKS * K], FP32)
            nc.scalar.sqrt(norms[:], norm2[:])
            weighted = pp.tile([128, CHUNKS * K], FP32)
            nc.vector.tensor_tensor(
                out=weighted[:], in0=norms[:], in1=S_sb[:], op=mybir.AluOpType.mult
            )
            res = pp.tile([128, 1], FP32)
            nc.vector.reduce_sum(res[:], weighted[:], axis=mybir.AxisListType.X)
            nc.sync.dma_start(out_d[:, :], res[:])

    nc.compile()
    return nc


def get_program(repeat=1):
    if repeat not in _PROGRAM_CACHE:
        _PROGRAM_CACHE[repeat] = _build_program(repeat)
    return _PROGRAM_CACHE[repeat]


def make_in_maps(P, i_indices, j_indices, S_vals):
    P = np.ascontiguousarray(np.asarray(P, dtype=np.float32))
    if TABLE_DT is not FP32:
        P = P.astype(mybir.dt.np(TABLE_DT))
    i_idx = np.asarray(i_indices).astype(np.int32)
    j_idx = np.asarray(j_indices).astype(np.int32)
    S = np.asarray(S_vals, dtype=np.float32)
    in_maps = []
    for core in range(N_CORES):
        b0 = core * B_PER_CORE
        i_c = i_idx[b0 : b0 + B_PER_CORE]            # [512]
        j_c = j_idx[b0 : b0 + B_PER_CORE]            # [512, 64]
        S_c = S[b0 : b0 + B_PER_CORE]                # [512, 64]
        # sort each b's (j, S) pairs by j: the penalty is a flat sum over
        # (b, k), so any per-b co-permutation is exact; ascending row ids
        # give the gather's descriptors better HBM locality.
        order = np.argsort(j_c, axis=1)
        j_c = np.take_along_axis(j_c, order, axis=1)
        S_c = np.take_along_axis(S_c, order, axis=1)
        # per half-chunk (c, h): slots [i_b, j_{b, h*KH .. h*KH+KH-1}];
        # partition p <-> b = c*128+p
        KH = K // K_SPLIT
        j_r = j_c.reshape(CHUNKS, BC, K_SPLIT, KH)
        i_r = np.broadcast_to(
            i_c.reshape(CHUNKS, BC)[:, :, None, None], (CHUNKS, BC, K_SPLIT, 1)
        )
        ij = np.concatenate([i_r, j_r], axis=3)          # [CHUNKS, BC, K_SPLIT, KH+1]
        idxIJ = np.ascontiguousarray(
            ij.transpose(1, 0, 2, 3).reshape(BC, CHUNKS * K_SPLIT * (KH + 1))
        )
        S_arr = np.ascontiguousarray(
            S_c.reshape(CHUNKS, BC, K_SPLIT, KH).transpose(1, 0, 2, 3)
            .reshape(BC, CHUNKS * K)
        )
        in_maps.append({"P": P, "idxIJ": idxIJ, "S": S_arr})
    return in_maps


def run_hw(in_maps, trace=False, repeat=1):
    nc = get_program(repeat)
    return bass_utils.run_bass_kernel_spmd(
        nc,
        in_maps,
        core_ids=list(range(N_CORES)),
        trace=trace,
    )


def kernel(P, i_indices, j_indices, S_vals):
    in_maps = make_in_maps(P, i_indices, j_indices, S_vals)
    res = run_hw(in_maps, trace=False)
    total = 0.0
    for core in range(N_CORES):
        total += float(np.asarray(res.results[core]["out"], dtype=np.float64).sum())
    return np.float32(total)



# revision 14
# speedup vs baseline: 1.5518x; 1.0654x over previous
"""Trainium2 Bass kernel for similarity-matrix penalty (gnn message passing).

penalty = sum_{b,k} S[b,k] * || P[i_b] - P[j_{b,k}] ||_2

Strategy (per the batch-sharding hint):
  - Shard the batch dim B=4096 across 8 cores (512 b's each); P is
    replicated to every core's HBM.
  - Per core, 4 chunks of 128 b's, each split into K_SPLIT=2 gathers.
    Gather layout: partition p <-> b, free dim = 33 row-slots x 128 d
    (slot 0 = P[i_b], slots 1..32 = P[j_{b,k}] for half the k's).
  - ONE indirect DMA per gather fetches all 33 rows per partition
    (4224 indices) instead of one call per k. The dest AP MUST be 2D
    ([128, KHI*D]); a 3D dest AP makes the SWDGE ucode emit only KHI
    descriptors. The gather casts fp32->bf16 in the SDMA datapath
    (SWDGE cast), halving SBUF write traffic and enabling the DVE 2x
    packed mode for the subtract.
  - K_SPLIT=2 x WP_BUFS=6 keeps ~5 indirect DMAs in flight, spreading
    descriptors across more SDMA engine rings: measured 100us -> 71us
    per iteration (HW, For_i repeat-difference, R=1001).
  - Each b's (j, S) pairs are sorted by j on the host: the penalty is
    a flat sum over (b, k), so any per-b co-permutation is exact, and
    ascending row addresses give the gather better HBM locality (~2us).
  - diff on DVE (bf16 tensor_tensor subtract with a stride-0
    broadcast AP of row slot 0, in-place), ACT square, DVE grouped
    tensor_reduce -> norm2; sqrt on ACT, multiply by S, free-dim
    reduce -> [128,1] per core. Host adds the 8x128 partials.
  - Measured: 70.7us/iter on 8 trn2 cores, rel err 4.0e-05 (fp32
    table; bf16 only in the SDMA cast + on-chip compute).
"""

import contextlib

import numpy as np

import concourse.bass as bass
import concourse.mybir as mybir
import concourse.tile as tile
from concourse import bacc
from concourse import bass_utils

N_ROWS = 500000
D = 128
B = 4096
K = 64
N_CORES = 8
B_PER_CORE = B // N_CORES      # 512
CHUNKS = 4
BC = B_PER_CORE // CHUNKS      # 128 b's per chunk (one per partition)
KI = K + 1                     # row slots per partition per chunk (i + 64 j's)
FP32 = mybir.dt.float32
BF16 = mybir.dt.bfloat16
I32 = mybir.dt.int32

# chunk indices whose square+reduce runs fused on ACT (per-k accum);
# the others use ACT square + DVE grouped reduce
ACT_CHUNKS = ()
WP_BUFS = 6
GATHER_DT = BF16
TABLE_DT = FP32       # dtype of P as staged in DRAM (host pre-casts if not fp32)
GATHER_ONLY = False   # diagnostic: skip all per-chunk compute
K_SPLIT = 2           # split each chunk's K dim into this many gathers
REDUCE_TREE = True    # pairwise-add d-halves (bf16 2x) before the reduce
FUSED_TAIL = False    # sqrt + *S per gather (shrinks the post-loop tail)

_PROGRAM_CACHE = {}


def _build_program(repeat=1):
    nc = bacc.Bacc(
        "TRN2",
        debug=False,
        enable_asserts=False,
        target_bir_lowering=False,
        num_devices=N_CORES,
        dynamic_dma_scratch_size=65536,
    )

    P_d = nc.dram_tensor("P", [N_ROWS, D], TABLE_DT, kind="ExternalInput")
    n_idx = CHUNKS * K_SPLIT * (K // K_SPLIT + 1)
    idx_d = nc.dram_tensor("idxIJ", [128, n_idx], I32, kind="ExternalInput")
    S_d = nc.dram_tensor("S", [128, CHUNKS * K], FP32, kind="ExternalInput")
    out_d = nc.dram_tensor("out", [128, 1], FP32, kind="ExternalOutput")

    with tile.TileContext(nc) as tc:
        with (
            tc.tile_pool(name="persist", bufs=1) as pp,
            tc.tile_pool(name="work", bufs=WP_BUFS) as wp,
        ):
            idx_sb = pp.tile([128, n_idx], I32)
            nc.sync.dma_start(idx_sb[:], idx_d[:, :])
            S_sb = pp.tile([128, CHUNKS * K], FP32)
            nc.sync.dma_start(S_sb[:], S_d[:, :])

            norm2 = pp.tile([128, CHUNKS * K], FP32)
            if FUSED_TAIL:
                norms_ft = pp.tile([128, CHUNKS * K], FP32)
                weighted_ft = pp.tile([128, CHUNKS * K], FP32)

            KH = K // K_SPLIT          # j rows per gather
            KHI = KH + 1               # +1 for the i row
            NCH = CHUNKS * K_SPLIT     # total gathers per rep
            # repeat>1 benches the body in a hardware loop: constant program
            # size for any R (the body is idempotent, norm2 rewritten each
            # iteration), so test.py's repeat-difference can use R >> 1.
            rep_ctx = tc.For_i(0, repeat) if repeat > 1 else contextlib.nullcontext(0)
            with rep_ctx:
              for ch in range(NCH):
                # big[p, 0, :] <- P[i_b]; big[p, 1+kk, :] <- P[j_{b,k}]
                # for b = (ch//K_SPLIT)*128+p, k = (ch%K_SPLIT)*KH+kk,
                # all in ONE indirect DMA, cast by the SDMA datapath.
                big = wp.tile([128, KHI, D], GATHER_DT)
                nc.gpsimd.indirect_dma_start(
                    out=big[:, :, :].rearrange("p a b -> p (a b)"),
                    out_offset=None,
                    in_=P_d[:, :],
                    in_offset=bass.IndirectOffsetOnAxis(
                        ap=idx_sb[:, ch * KHI : (ch + 1) * KHI], axis=0
                    ),
                )
                if GATHER_ONLY:
                    continue
                # diff on DVE (broadcast row slot 0 along k), in place
                nc.vector.tensor_tensor(
                    out=big[:, 1:KHI, :],
                    in0=big[:, 1:KHI, :],
                    in1=big[:, 0:1, :].to_broadcast((128, KH, D)),
                    op=mybir.AluOpType.subtract,
                )
                if ch in ACT_CHUNKS:
                    # fused square+sum on ACT, one inst per k
                    for k in range(KH):
                        nc.scalar.activation(
                            out=big[:, 1 + k, :],
                            in_=big[:, 1 + k, :],
                            func=mybir.ActivationFunctionType.Square,
                            accum_out=norm2[:, ch * KH + k : ch * KH + k + 1],
                        )
                elif REDUCE_TREE:
                    nc.scalar.square(big[:, 1:KHI, :], big[:, 1:KHI, :])
                    half = wp.tile([128, KH, D // 2], GATHER_DT)
                    nc.vector.tensor_tensor(
                        out=half[:],
                        in0=big[:, 1:KHI, 0 : D // 2],
                        in1=big[:, 1:KHI, D // 2 : D],
                        op=mybir.AluOpType.add,
                    )
                    nc.vector.reduce_sum(
                        norm2[:, ch * KH : (ch + 1) * KH],
                        half[:],
                        axis=mybir.AxisListType.X,
                    )
                else:
                    # square on ACT, grouped free-dim reduce on DVE
                    nc.scalar.square(big[:, 1:KHI, :], big[:, 1:KHI, :])
                    nc.vector.reduce_sum(
                        norm2[:, ch * KH : (ch + 1) * KH],
                        big[:, 1:KHI, :],
                        axis=mybir.AxisListType.X,
                    )
                if FUSED_TAIL:
                    # finish this gather's scalars now so the post-loop
                    # tail is just one grouped reduce + the out DMA
                    sl = slice(ch * KH, (ch + 1) * KH)
                    nc.scalar.sqrt(norms_ft[:, sl], norm2[:, sl])
                    nc.vector.tensor_tensor(
                        out=weighted_ft[:, sl],
                        in0=norms_ft[:, sl],
                        in1=S_sb[:, sl],
                        op=mybir.AluOpType.mult,
                    )

            if FUSED_TAIL and not GATHER_ONLY:
                res = pp.tile([128, 1], FP32)
                nc.vector.reduce_sum(
                    res[:], weighted_ft[:], axis=mybir.AxisListType.X
                )
                nc.sync.dma_start(out_d[:, :], res[:])
            else:
                if GATHER_ONLY:
                    nc.vector.memset(norm2[:], 0.0)
                # finals: sqrt -> *S -> free-dim reduce -> DRAM
                norms = pp.tile([128, CHUNKS * K], FP32)
                nc.scalar.sqrt(norms[:], norm2[:])
                weighted = pp.tile([128, CHUNKS * K], FP32)
                nc.vector.tensor_tensor(
                    out=weighted[:], in0=norms[:], in1=S_sb[:], op=mybir.AluOpType.mult
                )
                res = pp.tile([128, 1], FP32)
                nc.vector.reduce_sum(res[:], weighted[:], axis=mybir.AxisListType.X)
                nc.sync.dma_start(out_d[:, :], res[:])

    nc.compile()
    return nc


def get_program(repeat=1):
    if repeat not in _PROGRAM_CACHE:
        _PROGRAM_CACHE[repeat] = _build_program(repeat)
    return _PROGRAM_CACHE[repeat]


def make_in_maps(P, i_indices, j_indices, S_vals):
    P = np.ascontiguousarray(np.asarray(P, dtype=np.float32))
    if TABLE_DT is not FP32:
        P = P.astype(mybir.dt.np(TABLE_DT))
    i_idx = np.asarray(i_indices).astype(np.int32)
    j_idx = np.asarray(j_indices).astype(np.int32)
    S = np.asarray(S_vals, dtype=np.float32)
    in_maps = []
    for core in range(N_CORES):
        b0 = core * B_PER_CORE
        i_c = i_idx[b0 : b0 + B_PER_CORE]            # [512]
        j_c = j_idx[b0 : b0 + B_PER_CORE]            # [512, 64]
        S_c = S[b0 : b0 + B_PER_CORE]                # [512, 64]
        # sort each b's (j, S) pairs by j: the penalty is a flat sum over
        # (b, k), so any per-b co-permutation is exact; ascending row ids
        # give the gather's descriptors better HBM locality.
        order = np.argsort(j_c, axis=1)
        j_c = np.take_along_axis(j_c, order, axis=1)
        S_c = np.take_along_axis(S_c, order, axis=1)
        # per half-chunk (c, h): slots [i_b, j_{b, h*KH .. h*KH+KH-1}];
        # partition p <-> b = c*128+p
        KH = K // K_SPLIT
        j_r = j_c.reshape(CHUNKS, BC, K_SPLIT, KH)
        i_r = np.broadcast_to(
            i_c.reshape(CHUNKS, BC)[:, :, None, None], (CHUNKS, BC, K_SPLIT, 1)
        )
        ij = np.concatenate([i_r, j_r], axis=3)          # [CHUNKS, BC, K_SPLIT, KH+1]
        idxIJ = np.ascontiguousarray(
            ij.transpose(1, 0, 2, 3).reshape(BC, CHUNKS * K_SPLIT * (KH + 1))
        )
        S_arr = np.ascontiguousarray(
            S_c.reshape(CHUNKS, BC, K_SPLIT, KH).transpose(1, 0, 2, 3)
            .reshape(BC, CHUNKS * K)
        )
        in_maps.append({"P": P, "idxIJ": idxIJ, "S": S_arr})
    return in_maps


def run_hw(in_maps, trace=False, repeat=1):
    nc = get_program(repeat)
    return bass_utils.run_bass_kernel_spmd(
        nc,
        in_maps,
        core_ids=list(range(N_CORES)),
        trace=trace,
    )


def kernel(P, i_indices, j_indices, S_vals):
    in_maps = make_in_maps(P, i_indices, j_indices, S_vals)
    res = run_hw(in_maps, trace=False)
    total = 0.0
    for core in range(N_CORES):
        total += float(np.asarray(res.results[core]["out"], dtype=np.float64).sum())
    return np.float32(total)

